# revision 1
# baseline (speedup 1.0000x reference)
"""Bahdanau-attention kernel for Trainium2, 8-core data-parallel over batch.

Problem: context = softmax(w2 . tanh(enc @ W1_enc + hid @ W1_hid + b1)) @ enc
  B=32, S=2048, D=1024.  Each of the 8 cores handles 4 batch elements.

Device-side strategy (per core, per batch b, per 512-wide seq chunk):
  - encT [D, S] (host-transposed) slices feed the big matmul
    h^T[m-chunk] = sum_k W1_enc[k,m]^T @ encT[k]   (PE, PSUM f32 accum)
  - tanh+bias via ACT per-partition bias z[m] = (hid @ W1_hid + b1)[m-chunk]
  - e-scores in row form: e_row[1, 512] = sum_m w2[m]^T @ h^T[m]  (PE),
    emitted one m-step behind the h matmuls so the PE never waits on ACT
  - p_row = exp(e_row) on ACT with fused accum_out giving the softmax
    normalizer partial (no max subtraction: |e| <= sum|w2| ~ 26, safe fp32)
  - p broadcast to 128 partitions via K=1 ones-matmul on PE; exp/broadcast/
    context work for chunk i is emitted inside chunk i+1's matmul stream so
    the PE pipeline stays dense
  - ctx contribution on the otherwise-idle DVE: one broadcast multiply +
    one grouped reduce over the already-resident encT tile (the natural
    layout enc copy is never loaded; halves HBM traffic)
  - context[b] = ctx * (1/Z), 1/Z partition-broadcast on GPSIMD

Heavy matmuls run as float32r: 1 row/cycle on the PE at moving-N >= 256
(4x faster than float32), measured ~1.3e-4 matmul rel err.  float32r ISA
restrictions: producers of matmul operands must emit f32r-rounded data
(DMA + ACT do), moving free count and PSUM dst free count must be even,
dst 8B-aligned at partition 0 - hence the 2-column padding of w2.
"""

import numpy as np
from contextlib import ExitStack

import concourse.bacc as bacc
import concourse.tile as tile
from concourse import mybir
from concourse.bass_utils import run_bass_kernel_spmd

AFT = mybir.ActivationFunctionType
ALU = mybir.AluOpType
F32 = mybir.dt.float32

B, S, D = 32, 2048, 1024
NCORES = 8
BL = B // NCORES          # 4 batch elements per core
P = 128
KC = D // P               # 8 contraction / output chunks
S_SUB = 512               # seq chunk processed per inner iteration
NSS = S // S_SUB          # 4

# dtype used on the PE for the heavy matmuls
DT = mybir.dt.float32r


def _body(ctx, tc, encT, hidT, w1e, w1h, b1, w2, onesr, out):
    nc = tc.nc
    const = ctx.enter_context(tc.tile_pool(name="const", bufs=1))
    wpool = ctx.enter_context(tc.tile_pool(name="wpool", bufs=1))
    epool = ctx.enter_context(tc.tile_pool(name="epool", bufs=3))
    spool2 = ctx.enter_context(tc.tile_pool(name="spool2", bufs=2))
    hpool = ctx.enter_context(tc.tile_pool(name="hpool", bufs=4))
    spool = ctx.enter_context(tc.tile_pool(name="spool", bufs=2))
    cpool = ctx.enter_context(tc.tile_pool(name="cpool", bufs=2 * NSS + 2))
    # PSUM budget (8 banks): hp/zp 3 + e_row 2 + p_bc 2
    ppa = ctx.enter_context(tc.tile_pool(name="ppa", bufs=4, space="PSUM"))
    ppe = ctx.enter_context(tc.tile_pool(name="ppe", bufs=2, space="PSUM"))
    ppb = ctx.enter_context(tc.tile_pool(name="ppb", bufs=2, space="PSUM"))

    # --- phase 0 DMA order matters for startup: the z matmuls are first in
    # the PE stream, so their inputs (w1h, hidT) go first; then w1e
    # interleaved with the first chunk's encT slices; small consts last.
    w1e_t, w1h_t, hid_t, b1_t, w2_t = [], [], [], [], []
    for k in range(KC):
        t = wpool.tile([P, D], F32, name=f"w1h_{k}")
        nc.sync.dma_start(t[:], w1h[k * P:(k + 1) * P, :])
        w1h_t.append(t)
        t = const.tile([P, BL], F32, name=f"hid_{k}")
        nc.sync.dma_start(t[:], hidT[k * P:(k + 1) * P, :])
        hid_t.append(t)
    et0 = epool.tile([P, KC * S_SUB], DT, name="et_big", tag="et_big")
    for k in range(KC):
        t = wpool.tile([P, D], DT, name=f"w1e_{k}")
        nc.sync.dma_start(t[:], w1e[k * P:(k + 1) * P, :])
        w1e_t.append(t)
        nc.sync.dma_start(et0[:, k * S_SUB:(k + 1) * S_SUB],
                          encT[0, k * P:(k + 1) * P, 0:S_SUB])
    for k in range(KC):
        t = const.tile([P, 1], F32, name=f"b1_{k}")
        nc.sync.dma_start(t[:], b1[k * P:(k + 1) * P, :])
        b1_t.append(t)
        t = const.tile([P, 2], DT, name=f"w2_{k}")
        nc.sync.dma_start(t[:], w2[k * P:(k + 1) * P, :])
        w2_t.append(t)
    onesr_t = const.tile([1, P], DT, name="onesr_t")
    nc.sync.dma_start(onesr_t[:], onesr[:])

    # per-batch bias z = hid @ W1_hid + b1
    z_sb = []
    for m in range(KC):
        zp = ppa.tile([P, BL], F32, name="zp", tag="ppa_t")
        for k in range(KC):
            nc.tensor.matmul(
                zp[:], lhsT=w1h_t[k][:, m * P:(m + 1) * P], rhs=hid_t[k][:],
                start=(k == 0), stop=(k == KC - 1))
        zt = const.tile([P, BL], F32, name=f"z_{m}")
        nc.vector.tensor_scalar_add(zt[:], zp[:], b1_t[m][:])
        z_sb.append(zt)

    # --- pipelined main loop ---
    state = {}    # per-batch: z_parts tile + list of per-chunk ctx tiles
    pending = None  # chunk awaiting exp/broadcast/ctx emission

    def emit_post(pend):
        """exp, p-broadcast, and DVE context work for a finished chunk."""
        pb, pss, e_ps, et_big = pend
        st = state[pb]
        p_row = spool.tile([1, S_SUB], DT, name="p_row", tag="p_row")
        nc.scalar.activation(p_row[:], e_ps[0:1, :], AFT.Exp,
                             accum_out=st["z_parts"][0:1, pss:pss + 1])
        p_bc = ppb.tile([P, S_SUB], F32, name="p_bc")
        nc.tensor.matmul(p_bc[:], lhsT=onesr_t[:], rhs=p_row[:],
                         start=True, stop=True)
        scratch = spool2.tile([P, KC * S_SUB], F32, name="scr", tag="scr")
        p_rep = p_bc[:].rearrange("p (o s) -> p o s",
                                  o=1).broadcast_to((P, KC, S_SUB))
        nc.vector.tensor_tensor(
            scratch[:].rearrange("p (k s) -> p k s", k=KC),
            et_big[:].bitcast(F32).rearrange("p (k s) -> p k s", k=KC),
            p_rep, ALU.mult)
        cred = cpool.tile([P, KC], F32, name="cred", tag="cred")
        nc.vector.tensor_reduce(
            cred[:], scratch[:].rearrange("p (k s) -> p k s", k=KC),
            axis=mybir.AxisListType.X, op=ALU.add)
        st["creds"].append(cred)
        if pss == NSS - 1:
            emit_finalize(pb)

    def emit_finalize(pb):
        """context[pb] = (sum of chunk contributions) / Z, then store."""
        st = state.pop(pb)
        zsum = spool.tile([1, 1], F32, name="zsum", tag="zsum")
        nc.vector.tensor_reduce(zsum[:], st["z_parts"][:],
                                axis=mybir.AxisListType.X, op=ALU.add)
        zr = spool.tile([1, 1], F32, name="zr", tag="zr")
        nc.vector.reciprocal(zr[:], zsum[:])
        zr_bc = spool.tile([P, 1], F32, name="zr_bc", tag="zr_bc")
        nc.gpsimd.partition_broadcast(zr_bc[:], zr[:])
        creds = st["creds"]
        ctx_fin = cpool.tile([P, KC], F32, name="ctx_fin", tag="ctx_fin")
        nc.vector.tensor_tensor(ctx_fin[:], creds[0][:], creds[1][:], ALU.add)
        nc.vector.tensor_tensor(ctx_fin[:], ctx_fin[:], creds[2][:], ALU.add)
        nc.vector.tensor_tensor(ctx_fin[:], ctx_fin[:], creds[3][:], ALU.add)
        ctx_sc = cpool.tile([P, KC], F32, name="ctx_sc", tag="ctx_sc")
        nc.vector.tensor_scalar_mul(ctx_sc[:], ctx_fin[:], zr_bc[:])
        nc.sync.dma_start(out[pb].rearrange("(k p) -> p k", p=P), ctx_sc[:])

    for ci, (b, ss) in enumerate([(b, ss) for b in range(BL)
                                  for ss in range(NSS)]):
        if ss == 0:
            state[b] = {
                "z_parts": spool.tile([1, NSS], F32, name="z_parts",
                                      tag="z_parts"),
                "creds": [],
            }
        if ci == 0:
            et_big = et0
        else:
            et_big = epool.tile([P, KC * S_SUB], DT, name="et_big",
                                tag="et_big")
            for k in range(KC):
                nc.sync.dma_start(
                    et_big[:, k * S_SUB:(k + 1) * S_SUB],
                    encT[b, k * P:(k + 1) * P, ss * S_SUB:(ss + 1) * S_SUB])
        e_ps = ppe.tile([2, S_SUB], F32, name="e_ps")
        h_prev = None
        for m in range(KC):
            hp = ppa.tile([P, S_SUB], F32, name="hp", tag="ppa_t")
            for k in range(KC):
                nc.tensor.matmul(
                    hp[:], lhsT=w1e_t[k][:, m * P:(m + 1) * P],
                    rhs=et_big[:, k * S_SUB:(k + 1) * S_SUB],
                    start=(k == 0), stop=(k == KC - 1))
            if m == 0 and pending is not None:
                emit_post(pending)
                pending = None
            h_sb = hpool.tile([P, S_SUB], DT, name="h_sb", tag="h_sb")
            nc.scalar.activation(h_sb[:], hp[:], AFT.Tanh,
                                 bias=z_sb[m][:, b:b + 1])
            if h_prev is not None:
                nc.tensor.matmul(e_ps[:], lhsT=w2_t[m - 1][:], rhs=h_prev[:],
                                 start=(m == 1), stop=False)
            h_prev = h_sb
        nc.tensor.matmul(e_ps[:], lhsT=w2_t[KC - 1][:], rhs=h_prev[:],
                         start=False, stop=True)
        pending = (b, ss, e_ps, et_big)
    emit_post(pending)
    state.clear()


def build_program():
    nc = bacc.Bacc("TRN2", target_bir_lowering=False, debug=False,
                   num_devices=NCORES)
    encT = nc.dram_tensor("encT", [BL, D, S], DT, kind="ExternalInput").ap()
    hidT = nc.dram_tensor("hidT", [D, BL], F32, kind="ExternalInput").ap()
    w1e = nc.dram_tensor("w1e", [D, D], DT, kind="ExternalInput").ap()
    w1h = nc.dram_tensor("w1h", [D, D], F32, kind="ExternalInput").ap()
    b1 = nc.dram_tensor("b1", [D, 1], F32, kind="ExternalInput").ap()
    w2 = nc.dram_tensor("w2", [D, 2], DT, kind="ExternalInput").ap()
    onesr = nc.dram_tensor("onesr", [1, P], DT, kind="ExternalInput").ap()
    out = nc.dram_tensor("ctx_out", [BL, D], F32, kind="ExternalOutput").ap()
    with tile.TileContext(nc) as tc:
        with ExitStack() as ctx:
            _body(ctx, tc, encT, hidT, w1e, w1h, b1, w2, onesr, out)
    nc.compile()
    return nc


def prep_in_maps(inputs):
    enc = np.asarray(inputs["encoder_outputs"], dtype=np.float32)
    hid = np.asarray(inputs["hidden_state"], dtype=np.float32)
    W1 = np.asarray(inputs["W1"], dtype=np.float32)
    b1 = np.asarray(inputs["b1"], dtype=np.float32)
    w2 = np.asarray(inputs["w2"], dtype=np.float32)
    encT = np.ascontiguousarray(enc.transpose(0, 2, 1))
    w1e = np.ascontiguousarray(W1[:D])
    w1h = np.ascontiguousarray(W1[D:])
    b1c = np.ascontiguousarray(b1.reshape(D, 1))
    w2c = np.zeros((D, 2), dtype=np.float32)
    w2c[:, 0] = w2
    onesr_np = np.ones((1, P), dtype=np.float32)
    in_maps = []
    for c in range(NCORES):
        sl = slice(c * BL, (c + 1) * BL)
        in_maps.append({
            "encT": encT[sl],
            "hidT": np.ascontiguousarray(hid[sl].T),
            "w1e": w1e,
            "w1h": w1h,
            "b1": b1c,
            "w2": w2c,
            "onesr": onesr_np,
        })
    return in_maps


_NC_CACHE = None


def kernel(**inputs):
    global _NC_CACHE
    if _NC_CACHE is None:
        _NC_CACHE = build_program()
    nc = _NC_CACHE
    in_maps = prep_in_maps(inputs)
    res = run_bass_kernel_spmd(nc, in_maps, core_ids=list(range(NCORES)))
    out = np.empty((B, D), dtype=np.float32)
    for c in range(NCORES):
        out[c * BL:(c + 1) * BL] = res.results[c]["ctx_out"]
    return out



# revision 8
# speedup vs baseline: 1.8085x; 1.8085x over previous
"""Bahdanau-attention kernel for Trainium2, 8-core data-parallel over batch.

Problem: context = softmax(w2 . tanh(enc @ W1_enc + hid @ W1_hid + b1)) @ enc
  B=32, S=2048, D=1024.  Each of the 8 cores handles 4 batch elements.

Device-side strategy (per core, per batch b, per 512-wide seq chunk):
  - the heavy enc @ W1_enc matmul runs in fp8-e4m3 with DoubleRow perf
    mode: both operands quantized to e4m3 (W1 pre-scaled by 64 into the
    e4m3 normal range), two 128-deep k-tiles contracted per PE pass.
    The 1/64 de-scale rides the ACT tanh's per-op scale for free.
  - chunks are processed in PAIRS sharing each stationary weight load
    (one LDWEIGHTS feeds two 512-wide moving streams) to amortize the
    DoubleRow weight-load overhead.
  - tanh+bias via ACT: h = tanh(hp/64 + z[m]), output bf16.  z is the
    per-batch bias (hid @ W1_hid + b1), computed on-device in bf16 and
    emitted inside pair 0's matmul stream so the PE never waits on the
    W1_hid DMA at startup.
  - e-scores: e_row[2, 512] += w2[m]^T @ h[m] in bf16 on the PE, one
    m-step behind the hp matmuls so the PE never waits on ACT.
  - p_row = exp(e_row) on ACT with fused accum_out giving the softmax
    normalizer partial (no max subtraction: |e| <= sum|w2| ~ 26, safe).
  - p broadcast to 128 partitions via K=1 ones-matmul (fp32r) on PE.
  - context partials on DVE via fused tensor_tensor_reduce over a
    separate bf16 copy of encT: one op per k-group does the multiply by
    p and the 512-wide reduction (half the DVE work of mult+reduce).
  - context[b] = ctx * (1/Z), 1/Z partition-broadcast on GPSIMD.

Quantization error budget (validated vs the fp32 reference on the
harness inputs): fp8 enc/W1 + bf16 h/w2/hid/W1h/enc-context ==> rel_err
~1.2e-2 against the 2e-2 gate.
"""

import numpy as np
import ml_dtypes
from contextlib import ExitStack

import concourse.bacc as bacc
import concourse.tile as tile
from concourse import mybir
from concourse.bass_utils import run_bass_kernel_spmd

AFT = mybir.ActivationFunctionType
ALU = mybir.AluOpType
F32 = mybir.dt.float32
F32R = mybir.dt.float32r
BF16 = mybir.dt.bfloat16
F8 = mybir.dt.float8e4
DR = mybir.MatmulPerfMode.DoubleRow

B, S, D = 32, 2048, 1024
NCORES = 8
BL = B // NCORES          # 4 batch elements per core
P = 128
KC = D // P               # 8 contraction / output chunks
KP = KC // 2              # 4 DoubleRow k-pairs
S_SUB = 512               # seq chunk processed per inner iteration
NSS = S // S_SUB          # 4
NPAIR = NSS // 2          # chunk pairs per batch
W1_SCALE = 64.0           # fp8 pre-scale on W1_enc (power of two)


def declare_io(nc, input_kind="ExternalInput"):
    t = {}
    t["encT8"] = nc.dram_tensor("encT8", [BL, D, S], F8, kind=input_kind).ap()
    t["encTb"] = nc.dram_tensor("encTb", [BL, D, S], BF16,
                                kind=input_kind).ap()
    t["hidT"] = nc.dram_tensor("hidT", [D, BL], BF16, kind=input_kind).ap()
    t["w1e8"] = nc.dram_tensor("w1e8", [D, D], F8, kind=input_kind).ap()
    t["w1h"] = nc.dram_tensor("w1h", [D, D], BF16, kind=input_kind).ap()
    t["b1"] = nc.dram_tensor("b1", [D, 1], F32, kind=input_kind).ap()
    t["w2"] = nc.dram_tensor("w2", [D, 2], BF16, kind=input_kind).ap()
    t["onesr"] = nc.dram_tensor("onesr", [1, P], F32R, kind=input_kind).ap()
    t["ctx_out"] = nc.dram_tensor("ctx_out", [BL, D], F32,
                                  kind="ExternalOutput").ap()
    return t


def _body(ctx, tc, t):
    nc = tc.nc
    const = ctx.enter_context(tc.tile_pool(name="const", bufs=1))
    wpool = ctx.enter_context(tc.tile_pool(name="wpool", bufs=1))
    epool = ctx.enter_context(tc.tile_pool(name="epool", bufs=4))
    bpool = ctx.enter_context(tc.tile_pool(name="bpool", bufs=6))
    hpool = ctx.enter_context(tc.tile_pool(name="hpool", bufs=6))
    spool = ctx.enter_context(tc.tile_pool(name="spool", bufs=2))
    s2pool = ctx.enter_context(tc.tile_pool(name="s2pool", bufs=2))
    cpool = ctx.enter_context(tc.tile_pool(name="cpool", bufs=2 * NSS + 2))
    # PSUM budget (8 banks): hp 4 + e_row 2 + (p_bc | zp) 2
    ppa = ctx.enter_context(tc.tile_pool(name="ppa", bufs=4, space="PSUM"))
    ppe = ctx.enter_context(tc.tile_pool(name="ppe", bufs=2, space="PSUM"))
    ppb = ctx.enter_context(tc.tile_pool(name="ppb", bufs=2, space="PSUM"))

    encT8, encTb = t["encT8"], t["encTb"]

    # --- phase 0 DMAs, one instruction per tensor (HWDGE costs ~625ns
    # of serialized overhead per DMA *instruction*, so batch via 3D
    # access patterns).  Order = first-use order on the PE: w1e + pair
    # 0's fp8 et feed the very first hp matmuls; hid/w1h are only
    # needed by the z block emitted between pair 0's m=1 and m=2;
    # consts before their first consumers; bf16 et last (first read at
    # pair 1's m=0).
    w1e_all = wpool.tile([P, KC, D], F8, name="w1e_all")
    nc.sync.dma_start(w1e_all[:],
                      t["w1e8"].rearrange("(k p) m -> p k m", p=P))

    et8_tiles, etb_tiles = {}, {}
    encT8r = [encT8[b].rearrange("(k p) s -> p k s", p=P)
              for b in range(BL)]
    encTbr = [encTb[b].rearrange("(k p) s -> p k s", p=P)
              for b in range(BL)]

    def dma_pair8(q):
        """fp8 et for both chunks of pair q (feeds the hp matmuls)."""
        b, sp = divmod(q, NPAIR)
        for ss in (2 * sp, 2 * sp + 1):
            e8 = epool.tile([P, KC, S_SUB], F8, name="et8", tag="et8")
            nc.sync.dma_start(
                e8[:], encT8r[b][:, :, ss * S_SUB:(ss + 1) * S_SUB])
            et8_tiles[(b, ss)] = e8

    def dma_pairb(q):
        """bf16 et for both chunks of pair q (feeds the DVE context)."""
        b, sp = divmod(q, NPAIR)
        for ss in (2 * sp, 2 * sp + 1):
            eb = bpool.tile([P, KC, S_SUB], BF16, name="etb", tag="etb")
            nc.sync.dma_start(
                eb[:], encTbr[b][:, :, ss * S_SUB:(ss + 1) * S_SUB])
            etb_tiles[(b, ss)] = eb

    dma_pair8(0)

    hid_all = const.tile([P, KC, BL], BF16, name="hid_all")
    nc.sync.dma_start(hid_all[:],
                      t["hidT"].rearrange("(k p) b -> p k b", p=P))
    w1h_all = wpool.tile([P, KC, D], BF16, name="w1h_all")
    nc.sync.dma_start(w1h_all[:],
                      t["w1h"].rearrange("(k p) m -> p k m", p=P))
    b1_all = const.tile([P, KC], F32, name="b1_all")
    nc.sync.dma_start(b1_all[:],
                      t["b1"].rearrange("(k p) o -> p (k o)", p=P))
    w2_all = const.tile([P, KC, 2], BF16, name="w2_all")
    nc.sync.dma_start(w2_all[:],
                      t["w2"].rearrange("(k p) c -> p k c", p=P))
    onesr_t = const.tile([1, P], F32R, name="onesr_t")
    nc.sync.dma_start(onesr_t[:], t["onesr"][:])
    dma_pairb(0)

    z_sb = []            # filled by emit_z_block during pair 0

    def emit_z_block():
        """z[m] = hid @ W1_hid[:, m-chunk] + b1, all 8 m.  Sits between
        pair 0's m=1 and m=2 in the PE stream so the W1_hid DMA has
        landed by the time the PE gets here."""
        for m in range(KC):
            zp = ppb.tile([P, BL], F32, name="zp", tag="ppb_t")
            for k in range(KC):
                nc.tensor.matmul(
                    zp[:], lhsT=w1h_all[:, k, m * P:(m + 1) * P],
                    rhs=hid_all[:, k, :], start=(k == 0),
                    stop=(k == KC - 1))
            zt = const.tile([P, BL], F32, name=f"z_{m}")
            nc.vector.tensor_scalar_add(zt[:], zp[:], b1_all[:, m:m + 1])
            z_sb.append(zt)

    state = {}           # per-batch: z_parts tile + per-chunk cred tiles
    pending = []         # chunks awaiting exp/broadcast/ctx emission

    def emit_post(pend):
        """exp, p-broadcast, and DVE context work for a finished chunk."""
        pb, pss, e_ps, etb = pend
        st = state[pb]
        p_row = spool.tile([1, S_SUB], F32R, name="p_row", tag="p_row")
        nc.scalar.activation(p_row[:], e_ps[0:1, :], AFT.Exp,
                             accum_out=st["z_parts"][0:1, pss:pss + 1])
        p_bc = ppb.tile([P, S_SUB], F32, name="p_bc", tag="ppb_t")
        nc.tensor.matmul(p_bc[:], lhsT=onesr_t[:], rhs=p_row[:],
                         start=True, stop=True)
        cred = cpool.tile([P, KC], F32, name="cred", tag="cred")
        for k in range(KC):
            scr = s2pool.tile([P, S_SUB], BF16, name="scr", tag="scr")
            nc.vector.affine_mul_reduce(
                out=scr[:], accum_out=cred[:, k:k + 1],
                in0=etb[:, k, :], in1=p_bc[:], scale=1.0, bias=0.0)
        st["creds"].append(cred)
        if pss == NSS - 1:
            emit_finalize(pb)

    def emit_finalize(pb):
        """context[pb] = (sum of chunk contributions) / Z, then store."""
        st = state.pop(pb)
        zsum = spool.tile([1, 1], F32, name="zsum", tag="zsum")
        nc.vector.tensor_reduce(zsum[:], st["z_parts"][:],
                                axis=mybir.AxisListType.X, op=ALU.add)
        zr = spool.tile([1, 1], F32, name="zr", tag="zr")
        nc.vector.reciprocal(zr[:], zsum[:])
        zr_bc = spool.tile([P, 1], F32, name="zr_bc", tag="zr_bc")
        nc.gpsimd.partition_broadcast(zr_bc[:], zr[:])
        creds = st["creds"]
        ctx_fin = cpool.tile([P, KC], F32, name="ctx_fin", tag="ctx_fin")
        nc.vector.tensor_tensor(ctx_fin[:], creds[0][:], creds[1][:], ALU.add)
        nc.vector.tensor_tensor(ctx_fin[:], ctx_fin[:], creds[2][:], ALU.add)
        nc.vector.tensor_tensor(ctx_fin[:], ctx_fin[:], creds[3][:], ALU.add)
        ctx_sc = cpool.tile([P, KC], F32, name="ctx_sc", tag="ctx_sc")
        nc.vector.tensor_scalar_mul(ctx_sc[:], ctx_fin[:], zr_bc[:])
        nc.sync.dma_start(t["ctx_out"][pb].rearrange("(k p) -> p k", p=P),
                          ctx_sc[:])

    # --- pipelined main loop over chunk pairs ---
    for q in range(BL * NPAIR):
        b, sp = divmod(q, NPAIR)
        ss0, ss1 = 2 * sp, 2 * sp + 1
        if sp == 0:
            state[b] = {
                "z_parts": spool.tile([1, NSS], F32, name="z_parts",
                                      tag="z_parts"),
                "creds": [],
            }
        et0 = et8_tiles.pop((b, ss0))
        et1 = et8_tiles.pop((b, ss1))
        e_ps0 = ppe.tile([2, S_SUB], F32, name="e_ps", tag="e_ps")
        e_ps1 = ppe.tile([2, S_SUB], F32, name="e_ps", tag="e_ps")
        hq = {}          # hp PSUM tiles awaiting tanh (pair-0 deferral)
        h_prev0 = h_prev1 = None

        def emit_tanh(m, hp0, hp1):
            h0 = hpool.tile([P, S_SUB], BF16, name="h_sb", tag="h_sb")
            nc.scalar.activation(h0[:], hp0[:], AFT.Tanh,
                                 bias=z_sb[m][:, b:b + 1],
                                 scale=1.0 / W1_SCALE)
            h1 = hpool.tile([P, S_SUB], BF16, name="h_sb", tag="h_sb")
            nc.scalar.activation(h1[:], hp1[:], AFT.Tanh,
                                 bias=z_sb[m][:, b:b + 1],
                                 scale=1.0 / W1_SCALE)
            return h0, h1

        def emit_escore(m_prev, hh0, hh1):
            nc.tensor.matmul(e_ps0[:], lhsT=w2_all[:, m_prev, :],
                             rhs=hh0[:], start=(m_prev == 0), stop=False)
            nc.tensor.matmul(e_ps1[:], lhsT=w2_all[:, m_prev, :],
                             rhs=hh1[:], start=(m_prev == 0), stop=False)

        for m in range(KC):
            if q == 0 and m == 2:
                # z block + the deferred tanh/e-score work for m=0,1
                emit_z_block()
                h0a, h1a = emit_tanh(0, *hq.pop(0))
                h0b, h1b = emit_tanh(1, *hq.pop(1))
                emit_escore(0, h0a, h1a)
                h_prev0, h_prev1 = h0b, h1b
            hp0 = ppa.tile([P, S_SUB], F32, name="hp", tag="ppa_t")
            hp1 = ppa.tile([P, S_SUB], F32, name="hp", tag="ppa_t")
            for kp in range(KP):
                lw = w1e_all[:, 2 * kp:2 * kp + 2, m * P:(m + 1) * P]
                nc.tensor.matmul(hp0[:], lhsT=lw,
                                 rhs=et0[:, 2 * kp:2 * kp + 2, :],
                                 start=(kp == 0), stop=(kp == KP - 1),
                                 perf_mode=DR)
                nc.tensor.matmul(hp1[:], lhsT=lw,
                                 rhs=et1[:, 2 * kp:2 * kp + 2, :],
                                 start=(kp == 0), stop=(kp == KP - 1),
                                 perf_mode=DR)
            if m == 0:
                if pending:
                    emit_post(pending.pop(0))
                if q + 1 < BL * NPAIR:
                    dma_pair8(q + 1)
            elif m == 1:
                if pending:
                    emit_post(pending.pop(0))
                if q + 1 < BL * NPAIR:
                    dma_pairb(q + 1)
            if q == 0 and m < 2:
                hq[m] = (hp0, hp1)    # tanh deferred until z exists
                continue
            h0, h1 = emit_tanh(m, hp0, hp1)
            if h_prev0 is not None:
                emit_escore(m - 1, h_prev0, h_prev1)
            h_prev0, h_prev1 = h0, h1
        nc.tensor.matmul(e_ps0[:], lhsT=w2_all[:, KC - 1, :],
                         rhs=h_prev0[:], start=False, stop=True)
        nc.tensor.matmul(e_ps1[:], lhsT=w2_all[:, KC - 1, :],
                         rhs=h_prev1[:], start=False, stop=True)
        pending.append((b, ss0, e_ps0, etb_tiles.pop((b, ss0))))
        pending.append((b, ss1, e_ps1, etb_tiles.pop((b, ss1))))
    for pend in pending:
        emit_post(pend)
    pending.clear()
    state.clear()


def build_program():
    nc = bacc.Bacc("TRN2", target_bir_lowering=False, debug=False,
                   num_devices=NCORES)
    t = declare_io(nc, input_kind="ExternalInput")
    with tile.TileContext(nc) as tc:
        with ExitStack() as ctx:
            _body(ctx, tc, t)
    nc.compile()
    return nc


def prep_in_maps(inputs):
    f8 = ml_dtypes.float8_e4m3
    bf = ml_dtypes.bfloat16
    enc = np.asarray(inputs["encoder_outputs"], dtype=np.float32)
    hid = np.asarray(inputs["hidden_state"], dtype=np.float32)
    W1 = np.asarray(inputs["W1"], dtype=np.float32)
    b1 = np.asarray(inputs["b1"], dtype=np.float32)
    w2 = np.asarray(inputs["w2"], dtype=np.float32)
    encT = enc.transpose(0, 2, 1)                 # [B, D, S] strided view
    encT8 = encT.astype(f8)
    encTb = encT.astype(bf)
    w1e8 = (W1[:D] * np.float32(W1_SCALE)).astype(f8)
    w1hb = W1[D:].astype(bf)
    b1c = np.ascontiguousarray(b1.reshape(D, 1))
    w2c = np.zeros((D, 2), dtype=bf)
    w2c[:, 0] = w2.astype(bf)
    onesr_np = np.ones((1, P), dtype=np.float32)
    in_maps = []
    for c in range(NCORES):
        sl = slice(c * BL, (c + 1) * BL)
        in_maps.append({
            "encT8": encT8[sl],
            "encTb": encTb[sl],
            "hidT": np.ascontiguousarray(hid[sl].T).astype(bf),
            "w1e8": w1e8,
            "w1h": w1hb,
            "b1": b1c,
            "w2": w2c,
            "onesr": onesr_np,
        })
    return in_maps


_NC_CACHE = None


def kernel(**inputs):
    global _NC_CACHE
    if _NC_CACHE is None:
        _NC_CACHE = build_program()
    nc = _NC_CACHE
    in_maps = prep_in_maps(inputs)
    res = run_bass_kernel_spmd(nc, in_maps, core_ids=list(range(NCORES)))
    out = np.empty((B, D), dtype=np.float32)
    for c in range(NCORES):
        out[c * BL:(c + 1) * BL] = res.results[c]["ctx_out"]
    return out
